# revision 20
# baseline (speedup 1.0000x reference)
"""Multi-head causal attention (B=2, S=2048, D=1024, H=16) on 8 TRN2 NeuronCores.

Sharding: core c handles batch b = c//4 and head-group g = c%4 (4 heads, 256 dims).
Each core computes Q/K/V projections for its head group from x[b], runs causal
attention per head, and applies its 256 rows of Wo, producing a partial [S, D]
output. The host sums the 4 head-group partials per batch.

Device algorithm (per core); matmul operands bf16, accumulation fp32 in PSUM:
  qT/kT = Wq_g^T @ x^T, stored [64*2, pair, S] (head dims on partitions)
  v     = x @ Wv_g, stored per 128-seq block with an appended ones column
  attention runs per head-pair with the two heads interleaved per 512-wide
  i-chunk (chunks processed widest-first):
    S^T[j,i] strips via matmul(lhsT=kT_block, rhs=qT_chunk); the two heads'
    matmuls are issued back-to-back on disjoint PE row groups (K=64 row
    pairing) so they run concurrently; diagonal strips narrowed to the
    causally-valid column range
    P~^T = exp(scale * S^T) (ScalarE, 2 strips per instruction), diagonal
    blocks masked with an upper-triangular 0/1 multiply
    O'^T[65, i] += V'_j^T @ P~^T_j  (PSUM accumulate; row 64 = softmax denom)
    per chunk, both heads: numerator/denominator copied out of PSUM, the
    denominators reciprocal'd lane-parallel via a DRAM reshape bounce, and
    O^T = num * recip broadcast (stride-0 DRAM read)
  y = O @ Wo_g (lhsT = O^T tiles), DMA out.

The exp stream makes ScalarE the pacing engine during attention, so
independent TensorE work is interleaved as "fillers" between strip groups:
V projection and pair-1 Q/K projections fill pair-0's window, and the output
projection (emitted per chunk as soon as its oT range is normalized) fills
pair-1's window.
"""

import os
from collections import deque

import ml_dtypes
import numpy as np

import concourse.bass as bass
import concourse.mybir as mybir
import concourse.tile as tile
from concourse.bass_utils import run_bass_kernel_spmd
from concourse.masks import make_upper_triangular

F32 = mybir.dt.float32
BF16 = mybir.dt.bfloat16

B, S, D, H = 2, 2048, 1024, 16
HD = 64                     # head dim
GH = 4                      # heads per core
GC = GH * HD                # 256 projection cols per core
P = 128
KD = D // P                 # 8 contraction chunks for projections
NSB = S // P                # 16 seq blocks
CHW = 512                   # i-chunk width
NCH = S // CHW              # 4 i-chunks
SCALE = HD ** -0.5

_NC_CACHE = None
LAST_RESULTS = None         # BassKernelResults of the most recent run (for test.py)


class _Fillers:
    """Queue of small emission closures (1-2 TensorE ops each) drained
    between attention strip groups to keep the PE busy while ScalarE
    works through the exp stream. Markers let the consumer force-drain
    the prefix a dependent phase needs."""

    def __init__(self):
        self.q = deque()

    def add(self, fn):
        self.q.append(fn)

    def add_marker(self, key):
        self.q.append(key)

    def _emit_one(self):
        item = self.q.popleft()
        if callable(item):
            item()
            return None
        return item

    def step(self, n):
        done = 0
        while done < n and self.q:
            if self._emit_one() is None:
                done += 1

    def drain_until(self, key):
        while self.q:
            if self._emit_one() == key:
                return

    def drain(self):
        while self.q:
            self._emit_one()


def _emit_pair_attention(tc, pair, pools, tensors, fillers, emit_outproj,
                         pre_chunk=None):
    nc = tc.nc
    ps_sc, ps_pv, dpool, ppool, npool, opool = pools
    qT, kT, v_sb, oT, trimask = tensors

    order = range(NCH) if pair == 0 else range(NCH - 1, -1, -1)
    for c in order:
        njb = 4 * c + 4
        if pre_chunk is not None:
            pre_chunk(c)
        pvacc0 = ps_pv.tile([HD + 1, CHW], F32, tag="pv0", name="pvacc0")
        pvacc1 = ps_pv.tile([HD + 1, CHW], F32, tag="pv1", name="pvacc1")
        pvacc = {0: pvacc0, 1: pvacc1}
        # strip tasks, heads interleaved so paired score matmuls are adjacent
        tasks = [(hp, jb) for jb in range(njb) for hp in (0, 1)]
        for g0 in range(0, len(tasks), 2):
            group = tasks[g0:g0 + 2]
            sc = ps_sc.tile([P, 2, CHW], F32, tag="sc")
            pt = ppool.tile([P, 2, CHW], BF16, tag="pt")
            for t, (hp, jb) in enumerate(group):
                bp = hp * HD
                tl = max(0, jb - 4 * c) * P
                nc.tensor.matmul(
                    sc[:, t, tl:],
                    kT[bp:bp + HD, pair, jb * P:(jb + 1) * P],
                    qT[bp:bp + HD, pair, c * CHW + tl:(c + 1) * CHW])
            tlg = max(0, group[0][1] - 4 * c) * P
            nc.scalar.activation(
                pt[:, :len(group), tlg:], sc[:, :len(group), tlg:],
                mybir.ActivationFunctionType.Exp, scale=SCALE)
            for t, (hp, jb) in enumerate(group):
                if jb >= 4 * c:               # diagonal block: causal mask
                    tl = (jb - 4 * c) * P
                    nc.vector.tensor_mul(
                        pt[:, t, tl:tl + P], pt[:, t, tl:tl + P], trimask)
            for t, (hp, jb) in enumerate(group):
                h = pair * 2 + hp
                tl = max(0, jb - 4 * c) * P
                nc.tensor.matmul(
                    pvacc[hp][:, tl:], v_sb[:, jb, h, :], pt[:, t, tl:],
                    start=(jb == 0), stop=(jb == njb - 1))
            fillers.step(7 if pair == 0 else 4)

        # per-chunk normalize for both heads: copy num/denom out of PSUM,
        # lane-parallel reciprocal via DRAM reshape, broadcast, multiply
        dden = dpool.tile([2, CHW], F32, tag="dden")
        onums = {}
        for hp in (0, 1):
            onum = opool.tile([HD + 1, CHW], F32, tag=f"on{hp}")
            nc.vector.tensor_copy(out=onum, in_=pvacc[hp])
            nc.sync.dma_start(
                out=dden[hp:hp + 1, :], in_=onum[HD:HD + 1, :])
            onums[hp] = onum
        nel = 2 * CHW // P                    # 8 elems/lane
        rv = npool.tile([P, nel], F32, tag="recp")
        nc.sync.dma_start(out=rv, in_=bass.AP(
            tensor=dden.tensor, offset=dden.offset, ap=[[nel, P], [1, nel]]))
        nc.vector.reciprocal(out=rv, in_=rv)
        drec = dpool.tile([2, CHW], F32, tag="drec")
        nc.sync.dma_start(out=bass.AP(
            tensor=drec.tensor, offset=drec.offset,
            ap=[[nel, P], [1, nel]]), in_=rv)
        for hp in (0, 1):
            bcr = npool.tile([HD, CHW], F32, tag="bcr")
            nc.sync.dma_start(out=bcr, in_=bass.AP(
                tensor=drec.tensor, offset=drec.offset + hp * CHW,
                ap=[[0, HD], [1, CHW]]))
            if hp == 0:
                nc.vector.tensor_mul(
                    oT[0:HD, pair, c * CHW:(c + 1) * CHW],
                    onums[hp][0:HD, :], bcr)
            else:
                tmp = npool.tile([HD, CHW], BF16, tag="otmp")
                nc.vector.tensor_mul(tmp, onums[hp][0:HD, :], bcr)
                nc.sync.dma_start(
                    out=oT[HD:P, pair, c * CHW:(c + 1) * CHW], in_=tmp)
        if emit_outproj is not None:
            emit_outproj(c)


def _emit(tc):
    nc = tc.nc
    xT = nc.dram_tensor("xT", [D, S], BF16, kind="ExternalInput")
    wq = nc.dram_tensor("wq", [D, GC], BF16, kind="ExternalInput")
    wk = nc.dram_tensor("wk", [D, GC], BF16, kind="ExternalInput")
    wv = nc.dram_tensor("wv", [D, GC], BF16, kind="ExternalInput")
    wo = nc.dram_tensor("wo", [GC, D], BF16, kind="ExternalInput")
    y = nc.dram_tensor("y", [S, D], F32, kind="ExternalOutput")

    xT_t = xT[:].rearrange("(o p) s -> p o s", p=P)      # [128, 8, S]
    wq_t = wq[:].rearrange("(o p) c -> p o c", p=P)      # [128, 8, 256]
    wk_t = wk[:].rearrange("(o p) c -> p o c", p=P)
    wv_t = wv[:].rearrange("(o p) c -> p o c", p=P)
    wo_t = wo[:].rearrange("(o p) n -> p o n", p=P)      # [128, 2, 1024]

    from contextlib import ExitStack

    with ExitStack() as top:
        persist = top.enter_context(tc.tile_pool(name="persist", bufs=1))

        trimask = persist.tile([P, P], BF16)             # 1.0 where j<=i else 0
        make_upper_triangular(nc, trimask, val=1.0, diag=True)
        ones_bf = persist.tile([P, 1], BF16)
        nc.vector.memset(ones_bf, 1.0)

        wq_sb = persist.tile([P, KD, GC], BF16)
        wk_sb = persist.tile([P, KD, GC], BF16)
        wv_sb = persist.tile([P, KD, GC], BF16)
        wo_sb = persist.tile([P, 2, D], BF16)
        xfull = persist.tile([P, KD, S], BF16)
        # first-needed slices first, split across the two HWDGE engines
        for k in range(KD):
            nc.sync.dma_start(
                out=xfull[:, k, 0:CHW], in_=xT_t[:, k, 0:CHW])
            nc.scalar.dma_start(out=wq_sb[:, k, :], in_=wq_t[:, k, :])
            nc.scalar.dma_start(out=wk_sb[:, k, :], in_=wk_t[:, k, :])
        nc.scalar.dma_start(out=wv_sb, in_=wv_t)
        for ch in range(1, NCH):
            for k in range(KD):
                if ch == NCH - 1:
                    eng = nc.gpsimd
                else:
                    eng = nc.sync if (ch + k) % 2 == 0 else nc.scalar
                eng.dma_start(
                    out=xfull[:, k, ch * CHW:(ch + 1) * CHW],
                    in_=xT_t[:, k, ch * CHW:(ch + 1) * CHW])
        nc.sync.dma_start(out=wo_sb, in_=wo_t)

        qT = persist.tile([P, 2, S], BF16)               # [pair-cols, pair, seq]
        kT = persist.tile([P, 2, S], BF16)
        v_sb = persist.tile([P, NSB, GH, HD + 1], BF16)  # ones col appended
        oT = persist.tile([P, 2, S], BF16)
        nc.vector.tensor_copy(
            out=v_sb[:, :, :, HD:HD + 1],
            in_=ones_bf[:, 0:1].to_broadcast((P, NSB, GH, 1)))

        tensors = (qT, kT, v_sb, oT, trimask)

        # ---- attention with all projections as ordered fillers ----
        with ExitStack() as ph_b:
            ps_sc = ph_b.enter_context(
                tc.tile_pool(name="ps_sc", bufs=2, space="PSUM"))
            ps_pv = ph_b.enter_context(
                tc.tile_pool(name="ps_pv", bufs=1, space="PSUM"))
            ps_fill = ph_b.enter_context(
                tc.tile_pool(name="ps_fill", bufs=2, space="PSUM"))
            dpool = ph_b.enter_context(
                tc.tile_pool(name="dscr", bufs=4, space="DRAM"))
            ppool = ph_b.enter_context(tc.tile_pool(name="pstrip", bufs=3))
            npool = ph_b.enter_context(tc.tile_pool(name="norm", bufs=4))
            opool = ph_b.enter_context(tc.tile_pool(name="onum", bufs=2))
            ypool = ph_b.enter_context(tc.tile_pool(name="ystage", bufs=2))
            pools = (ps_sc, ps_pv, dpool, ppool, npool, opool)

            f0 = _Fillers()

            def _proj_chunk(which, pair_, ch):
                # which: 0=Q, 1=K; emits 8 accumulating matmuls + copy-out
                cell = {}
                w_sb = wq_sb if which == 0 else wk_sb
                dst = qT if which == 0 else kT

                def alloc_mm(k, cell=cell, ch=ch, w_sb=w_sb, pair_=pair_):
                    if k == 0:
                        cell["p"] = ps_fill.tile(
                            [P, CHW], F32, tag="fill", name="fillqk")
                    nc.tensor.matmul(
                        cell["p"], w_sb[:, k, pair_ * P:(pair_ + 1) * P],
                        xfull[:, k, ch * CHW:(ch + 1) * CHW],
                        start=(k == 0), stop=(k == KD - 1))

                def copy(cell=cell, ch=ch, dst=dst, pair_=pair_):
                    nc.vector.tensor_copy(
                        out=dst[:, pair_, ch * CHW:(ch + 1) * CHW],
                        in_=cell["p"])

                for k in range(KD):
                    f0.add(lambda k=k: alloc_mm(k))
                f0.add(copy)

            def _v_block(sb):
                cell = {}

                def alloc_mm(k, cell=cell, sb=sb):
                    if k == 0:
                        cell["pv"] = ps_fill.tile(
                            [P, CHW], F32, tag="fill", name="fillpv")
                    nc.tensor.matmul(
                        cell["pv"][:, 0:GC],
                        xfull[:, k, sb * P:(sb + 1) * P], wv_sb[:, k, :],
                        start=(k == 0), stop=(k == KD - 1))

                def copy(cell=cell, sb=sb):
                    nc.vector.tensor_copy(
                        out=v_sb[:, sb, :, 0:HD],
                        in_=cell["pv"][:, 0:GC].rearrange(
                            "p (h d) -> p h d", h=GH))

                for k in range(KD):
                    f0.add(lambda k=k: alloc_mm(k))
                f0.add(copy)

            # pair-0 prerequisites per chunk, in ascending-chunk order
            for ch in range(NCH):
                _proj_chunk(0, 0, ch)
                _proj_chunk(1, 0, ch)
                for s4 in range(CHW // P):
                    _v_block(ch * (CHW // P) + s4)
                f0.add_marker(("pre0", ch))
            # pair-1 Q/K projections (consumed as pair-0 window fillers)
            for ch in range(NCH):
                _proj_chunk(0, 1, ch)
                _proj_chunk(1, 1, ch)
            f0.add_marker("qk1_done")

            def _pre0(c):
                f0.drain_until(("pre0", c))

            _emit_pair_attention(tc, 0, pools, tensors, f0, None,
                                 pre_chunk=_pre0)
            f0.drain_until("qk1_done")
            f0.drain()

            # pair-1 fillers: output projection per normalized chunk
            f1 = _Fillers()

            def _outproj_chunk(c):
                for s4 in range(CHW // P):
                    sb = c * (CHW // P) + s4
                    cell = {}

                    def alloc(cell=cell):
                        cell["ysb"] = ypool.tile(
                            [P, D], F32, tag="ysb", name="ysb")

                    f1.add(alloc)
                    for nch in range(2):
                        def mm(gc, cell=cell, sb=sb, nch=nch):
                            if gc == 0:
                                cell["py"] = ps_fill.tile(
                                    [P, CHW], F32, tag="fill", name="fillpy")
                            nc.tensor.matmul(
                                cell["py"], oT[:, gc, sb * P:(sb + 1) * P],
                                wo_sb[:, gc, nch * CHW:(nch + 1) * CHW],
                                start=(gc == 0), stop=(gc == 1))

                        def cp(cell=cell, nch=nch):
                            nc.vector.tensor_copy(
                                out=cell["ysb"][:, nch * CHW:(nch + 1) * CHW],
                                in_=cell["py"])

                        f1.add(lambda mm=mm: mm(0))
                        f1.add(lambda mm=mm: mm(1))
                        f1.add(cp)

                    def out_dma(cell=cell, sb=sb):
                        eng = nc.sync if sb % 2 == 0 else nc.scalar
                        eng.dma_start(
                            out=y[sb * P:(sb + 1) * P, :], in_=cell["ysb"])

                    f1.add(out_dma)

            _emit_pair_attention(tc, 1, pools, tensors, f1, _outproj_chunk)
            f1.drain()


def _fix_instruction_waits(nc):
    """Some lowered ISA structs (fp32r matmul LDW, DMA pseudo) carry at most
    one sync wait. Normalize: hoist excess waits onto NoOps inserted
    immediately before the instruction in the scheduled stream (same engine,
    so program order preserves the wait semantics)."""
    fixed = 0
    for blk in nc.m.functions[0].blocks:
        insts = blk.instructions
        idx = 0
        while idx < len(insts):
            inst = insts[idx]
            si = getattr(inst, "sync_info", None)
            if si is not None and len(si.on_wait) > 1:
                waits = list(si.on_wait)
                for j, wt in enumerate(waits[:-1]):
                    nop = mybir.InstNoOp(
                        name=f"I-wfix{fixed}-{j}-{inst.name}",
                        engine=inst.engine,
                        sync_info=mybir.SyncInfo(on_wait=[wt], on_update=[]))
                    insts.insert(idx, nop)
                    idx += 1
                inst.sync_info = mybir.SyncInfo(
                    on_wait=[waits[-1]], on_update=list(si.on_update))
                fixed += 1
            idx += 1
    return fixed


def _build():
    global _NC_CACHE
    if _NC_CACHE is None:
        nc = bass.Bass()
        with tile.TileContext(nc) as tc:
            _emit(tc)
        _fix_instruction_waits(nc)
        _NC_CACHE = nc
    return _NC_CACHE


def kernel(x, Wq, Wkv, Wo):
    global LAST_RESULTS
    x = np.asarray(x, dtype=np.float32)
    Wq = np.asarray(Wq, dtype=np.float32)
    Wkv = np.asarray(Wkv, dtype=np.float32)
    Wo = np.asarray(Wo, dtype=np.float32)

    nc = _build()
    bf = ml_dtypes.bfloat16
    in_maps = []
    for c in range(8):
        b, g = divmod(c, 4)
        cs = slice(GC * g, GC * (g + 1))
        in_maps.append({
            "xT": np.ascontiguousarray(x[b].T).astype(bf),
            "wq": np.ascontiguousarray(Wq[:, cs]).astype(bf),
            "wk": np.ascontiguousarray(Wkv[:, 0:D][:, cs]).astype(bf),
            "wv": np.ascontiguousarray(Wkv[:, D:2 * D][:, cs]).astype(bf),
            "wo": np.ascontiguousarray(Wo[cs, :]).astype(bf),
        })

    trace = os.environ.get("ATTN_KERNEL_TRACE", "0") == "1"
    res = run_bass_kernel_spmd(nc, in_maps, list(range(8)), trace=trace)
    LAST_RESULTS = res

    out = np.zeros((B, S, D), dtype=np.float32)
    for c in range(8):
        b = c // 4
        out[b] += res.results[c]["y"]
    return out


if __name__ == "__main__":
    rng = np.random.default_rng(0)
    s = 1.0 / np.sqrt(D)
    inputs = {
        "x": rng.standard_normal((B, S, D), dtype=np.float32),
        "Wq": rng.standard_normal((D, D), dtype=np.float32) * s,
        "Wkv": rng.standard_normal((D, 2 * D), dtype=np.float32) * s,
        "Wo": rng.standard_normal((D, D), dtype=np.float32) * s,
    }
    out = kernel(**inputs)
    print("out", out.shape, out.dtype, float(np.abs(out).mean()))


# revision 21
# speedup vs baseline: 1.0703x; 1.0703x over previous
"""Multi-head causal attention (B=2, S=2048, D=1024, H=16) on 8 TRN2 NeuronCores.

Sharding: core c handles batch b = c//4 and head-group g = c%4 (4 heads, 256 dims).
Each core computes Q/K/V projections for its head group from x[b], runs causal
attention per head, and applies its 256 rows of Wo, producing a partial [S, D]
output. The host sums the 4 head-group partials per batch.

Device algorithm (per core); matmul operands bf16, accumulation fp32 in PSUM:
  qT/kT = Wq_g^T @ x^T, stored [64*2, pair, S] (head dims on partitions)
  v     = x @ Wv_g, stored per 128-seq block with an appended ones column
  attention runs per head-pair with the two heads interleaved per 512-wide
  i-chunk (chunks processed widest-first):
    S^T[j,i] strips via matmul(lhsT=kT_block, rhs=qT_chunk); the two heads'
    matmuls are issued back-to-back on disjoint PE row groups (K=64 row
    pairing) so they run concurrently; diagonal strips narrowed to the
    causally-valid column range
    P~^T = exp(scale * S^T) (ScalarE, 2 strips per instruction), diagonal
    blocks masked with an upper-triangular 0/1 multiply
    O'^T[65, i] += V'_j^T @ P~^T_j  (PSUM accumulate; row 64 = softmax denom)
    per chunk, both heads: numerator/denominator copied out of PSUM, the
    denominators reciprocal'd lane-parallel via a DRAM reshape bounce, and
    O^T = num * recip broadcast (stride-0 DRAM read)
  y = O @ Wo_g (lhsT = O^T tiles), DMA out.

The exp stream makes ScalarE the pacing engine during attention, so
independent TensorE work is interleaved as "fillers" between strip groups:
V projection and pair-1 Q/K projections fill pair-0's window, and the output
projection (emitted per chunk as soon as its oT range is normalized) fills
pair-1's window.
"""

import os
from collections import deque

import ml_dtypes
import numpy as np

import concourse.bass as bass
import concourse.mybir as mybir
import concourse.tile as tile
from concourse.bass_utils import run_bass_kernel_spmd
from concourse.masks import make_upper_triangular

F32 = mybir.dt.float32
BF16 = mybir.dt.bfloat16

B, S, D, H = 2, 2048, 1024, 16
HD = 64                     # head dim
GH = 4                      # heads per core
GC = GH * HD                # 256 projection cols per core
P = 128
KD = D // P                 # 8 contraction chunks for projections
NSB = S // P                # 16 seq blocks
CHW = 512                   # i-chunk width
NCH = S // CHW              # 4 i-chunks
SCALE = HD ** -0.5

_NC_CACHE = None
LAST_RESULTS = None         # BassKernelResults of the most recent run (for test.py)


class _Fillers:
    """Queue of small emission closures (1-2 TensorE ops each) drained
    between attention strip groups to keep the PE busy while ScalarE
    works through the exp stream. Markers let the consumer force-drain
    the prefix a dependent phase needs."""

    def __init__(self):
        self.q = deque()

    def add(self, fn):
        self.q.append(fn)

    def add_marker(self, key):
        self.q.append(key)

    def _emit_one(self):
        item = self.q.popleft()
        if callable(item):
            item()
            return None
        return item

    def step(self, n):
        done = 0
        while done < n and self.q:
            if self._emit_one() is None:
                done += 1

    def drain_until(self, key):
        while self.q:
            if self._emit_one() == key:
                return

    def drain(self):
        while self.q:
            self._emit_one()


def _emit_pair_attention(tc, pair, pools, tensors, fillers, emit_outproj,
                         pre_chunk=None):
    nc = tc.nc
    ps_sc, ps_pv, dpool, ppool, npool, opool = pools
    qT, kT, v_sb, oT, trimask = tensors

    order = range(NCH) if pair == 0 else range(NCH - 1, -1, -1)
    for c in order:
        njb = 4 * c + 4
        if pre_chunk is not None:
            pre_chunk(c)
        pvacc0 = ps_pv.tile([HD + 1, CHW], F32, tag="pv0", name="pvacc0")
        pvacc1 = ps_pv.tile([HD + 1, CHW], F32, tag="pv1", name="pvacc1")
        pvacc = {0: pvacc0, 1: pvacc1}
        # strip tasks, heads interleaved so paired score matmuls are adjacent
        tasks = [(hp, jb) for jb in range(njb) for hp in (0, 1)]
        for g0 in range(0, len(tasks), 2):
            group = tasks[g0:g0 + 2]
            sc = ps_sc.tile([P, 2, CHW], F32, tag="sc")
            pt = ppool.tile([P, 2, CHW], BF16, tag="pt")
            for t, (hp, jb) in enumerate(group):
                bp = hp * HD
                tl = max(0, jb - 4 * c) * P
                nc.tensor.matmul(
                    sc[:, t, tl:],
                    kT[bp:bp + HD, pair, jb * P:(jb + 1) * P],
                    qT[bp:bp + HD, pair, c * CHW + tl:(c + 1) * CHW])
            tlg = max(0, group[0][1] - 4 * c) * P
            nc.scalar.activation(
                pt[:, :len(group), tlg:], sc[:, :len(group), tlg:],
                mybir.ActivationFunctionType.Exp, scale=SCALE)
            for t, (hp, jb) in enumerate(group):
                if jb >= 4 * c:               # diagonal block: causal mask
                    tl = (jb - 4 * c) * P
                    nc.vector.tensor_mul(
                        pt[:, t, tl:tl + P], pt[:, t, tl:tl + P], trimask)
            for t, (hp, jb) in enumerate(group):
                h = pair * 2 + hp
                tl = max(0, jb - 4 * c) * P
                nc.tensor.matmul(
                    pvacc[hp][:, tl:], v_sb[:, jb, h, :], pt[:, t, tl:],
                    start=(jb == 0), stop=(jb == njb - 1))
            fillers.step(7 if pair == 0 else 4)

        # per-chunk normalize for both heads: copy num/denom out of PSUM,
        # lane-parallel reciprocal via DRAM reshape, broadcast, multiply
        dden = dpool.tile([2, CHW], F32, tag="dden")
        onums = {}
        for hp in (0, 1):
            onum = opool.tile([HD + 1, CHW], F32, tag=f"on{hp}")
            nc.vector.tensor_copy(out=onum, in_=pvacc[hp])
            nc.sync.dma_start(
                out=dden[hp:hp + 1, :], in_=onum[HD:HD + 1, :])
            onums[hp] = onum
        nel = 2 * CHW // P                    # 8 elems/lane
        rv = npool.tile([P, nel], F32, tag="recp")
        nc.sync.dma_start(out=rv, in_=bass.AP(
            tensor=dden.tensor, offset=dden.offset, ap=[[nel, P], [1, nel]]))
        nc.vector.reciprocal(out=rv, in_=rv)
        drec = dpool.tile([2, CHW], F32, tag="drec")
        nc.sync.dma_start(out=bass.AP(
            tensor=drec.tensor, offset=drec.offset,
            ap=[[nel, P], [1, nel]]), in_=rv)
        for hp in (0, 1):
            bcr = npool.tile([HD, CHW], F32, tag="bcr")
            nc.sync.dma_start(out=bcr, in_=bass.AP(
                tensor=drec.tensor, offset=drec.offset + hp * CHW,
                ap=[[0, HD], [1, CHW]]))
            if hp == 0:
                nc.vector.tensor_mul(
                    oT[0:HD, pair, c * CHW:(c + 1) * CHW],
                    onums[hp][0:HD, :], bcr)
            else:
                tmp = npool.tile([HD, CHW], BF16, tag="otmp")
                nc.vector.tensor_mul(tmp, onums[hp][0:HD, :], bcr)
                nc.sync.dma_start(
                    out=oT[HD:P, pair, c * CHW:(c + 1) * CHW], in_=tmp)
        if emit_outproj is not None:
            emit_outproj(c)


def _emit(tc):
    nc = tc.nc
    xT = nc.dram_tensor("xT", [D, S], BF16, kind="ExternalInput")
    wq = nc.dram_tensor("wq", [D, GC], BF16, kind="ExternalInput")
    wk = nc.dram_tensor("wk", [D, GC], BF16, kind="ExternalInput")
    wv = nc.dram_tensor("wv", [D, GC], BF16, kind="ExternalInput")
    wo = nc.dram_tensor("wo", [GC, D], BF16, kind="ExternalInput")
    y = nc.dram_tensor("y", [S, D], F32, kind="ExternalOutput")

    xT_t = xT[:].rearrange("(o p) s -> p o s", p=P)      # [128, 8, S]
    wq_t = wq[:].rearrange("(o p) c -> p o c", p=P)      # [128, 8, 256]
    wk_t = wk[:].rearrange("(o p) c -> p o c", p=P)
    wv_t = wv[:].rearrange("(o p) c -> p o c", p=P)
    wo_t = wo[:].rearrange("(o p) n -> p o n", p=P)      # [128, 2, 1024]

    from contextlib import ExitStack

    with ExitStack() as top:
        persist = top.enter_context(tc.tile_pool(name="persist", bufs=1))

        trimask = persist.tile([P, P], BF16)             # 1.0 where j<=i else 0
        make_upper_triangular(nc, trimask, val=1.0, diag=True)
        ones_bf = persist.tile([P, 1], BF16)
        nc.vector.memset(ones_bf, 1.0)

        wq_sb = persist.tile([P, KD, GC], BF16)
        wk_sb = persist.tile([P, KD, GC], BF16)
        wv_sb = persist.tile([P, KD, GC], BF16)
        wo_sb = persist.tile([P, 2, D], BF16)
        xfull = persist.tile([P, KD, S], BF16)
        # first-needed slices first, split across the two HWDGE engines
        for k in range(KD):
            nc.sync.dma_start(
                out=xfull[:, k, 0:CHW], in_=xT_t[:, k, 0:CHW])
            nc.scalar.dma_start(out=wq_sb[:, k, :], in_=wq_t[:, k, :])
            nc.scalar.dma_start(out=wk_sb[:, k, :], in_=wk_t[:, k, :])
        nc.scalar.dma_start(out=wv_sb, in_=wv_t)
        for ch in range(1, NCH):
            for k in range(KD):
                eng = nc.sync if (ch + k) % 2 == 0 else nc.scalar
                eng.dma_start(
                    out=xfull[:, k, ch * CHW:(ch + 1) * CHW],
                    in_=xT_t[:, k, ch * CHW:(ch + 1) * CHW])
        nc.sync.dma_start(out=wo_sb, in_=wo_t)

        qT = persist.tile([P, 2, S], BF16)               # [pair-cols, pair, seq]
        kT = persist.tile([P, 2, S], BF16)
        v_sb = persist.tile([P, NSB, GH, HD + 1], BF16)  # ones col appended
        oT = persist.tile([P, 2, S], BF16)
        nc.vector.tensor_copy(
            out=v_sb[:, :, :, HD:HD + 1],
            in_=ones_bf[:, 0:1].to_broadcast((P, NSB, GH, 1)))

        tensors = (qT, kT, v_sb, oT, trimask)

        # ---- attention with all projections as ordered fillers ----
        with ExitStack() as ph_b:
            ps_sc = ph_b.enter_context(
                tc.tile_pool(name="ps_sc", bufs=2, space="PSUM"))
            ps_pv = ph_b.enter_context(
                tc.tile_pool(name="ps_pv", bufs=1, space="PSUM"))
            ps_fill = ph_b.enter_context(
                tc.tile_pool(name="ps_fill", bufs=2, space="PSUM"))
            dpool = ph_b.enter_context(
                tc.tile_pool(name="dscr", bufs=4, space="DRAM"))
            ppool = ph_b.enter_context(tc.tile_pool(name="pstrip", bufs=3))
            npool = ph_b.enter_context(tc.tile_pool(name="norm", bufs=4))
            opool = ph_b.enter_context(tc.tile_pool(name="onum", bufs=2))
            ypool = ph_b.enter_context(tc.tile_pool(name="ystage", bufs=2))
            pools = (ps_sc, ps_pv, dpool, ppool, npool, opool)

            f0 = _Fillers()

            def _proj_chunk(which, pair_, ch):
                # which: 0=Q, 1=K; emits 8 accumulating matmuls + copy-out
                cell = {}
                w_sb = wq_sb if which == 0 else wk_sb
                dst = qT if which == 0 else kT

                def alloc_mm(k, cell=cell, ch=ch, w_sb=w_sb, pair_=pair_):
                    if k == 0:
                        cell["p"] = ps_fill.tile(
                            [P, CHW], F32, tag="fill", name="fillqk")
                    nc.tensor.matmul(
                        cell["p"], w_sb[:, k, pair_ * P:(pair_ + 1) * P],
                        xfull[:, k, ch * CHW:(ch + 1) * CHW],
                        start=(k == 0), stop=(k == KD - 1))

                def copy(cell=cell, ch=ch, dst=dst, pair_=pair_):
                    nc.vector.tensor_copy(
                        out=dst[:, pair_, ch * CHW:(ch + 1) * CHW],
                        in_=cell["p"])

                for k in range(KD):
                    f0.add(lambda k=k: alloc_mm(k))
                f0.add(copy)

            def _v_block(sb):
                cell = {}

                def alloc_mm(k, cell=cell, sb=sb):
                    if k == 0:
                        cell["pv"] = ps_fill.tile(
                            [P, CHW], F32, tag="fill", name="fillpv")
                    nc.tensor.matmul(
                        cell["pv"][:, 0:GC],
                        xfull[:, k, sb * P:(sb + 1) * P], wv_sb[:, k, :],
                        start=(k == 0), stop=(k == KD - 1))

                def copy(cell=cell, sb=sb):
                    nc.vector.tensor_copy(
                        out=v_sb[:, sb, :, 0:HD],
                        in_=cell["pv"][:, 0:GC].rearrange(
                            "p (h d) -> p h d", h=GH))

                for k in range(KD):
                    f0.add(lambda k=k: alloc_mm(k))
                f0.add(copy)

            # pair-0 prerequisites per chunk, in ascending-chunk order
            for ch in range(NCH):
                _proj_chunk(0, 0, ch)
                _proj_chunk(1, 0, ch)
                for s4 in range(CHW // P):
                    _v_block(ch * (CHW // P) + s4)
                f0.add_marker(("pre0", ch))
            # pair-1 Q/K projections (consumed as pair-0 window fillers)
            for ch in range(NCH):
                _proj_chunk(0, 1, ch)
                _proj_chunk(1, 1, ch)
            f0.add_marker("qk1_done")

            def _pre0(c):
                f0.drain_until(("pre0", c))

            _emit_pair_attention(tc, 0, pools, tensors, f0, None,
                                 pre_chunk=_pre0)
            f0.drain_until("qk1_done")
            f0.drain()

            # pair-1 fillers: output projection per normalized chunk
            f1 = _Fillers()

            def _outproj_chunk(c):
                for s4 in range(CHW // P):
                    sb = c * (CHW // P) + s4
                    cell = {}

                    def alloc(cell=cell):
                        cell["ysb"] = ypool.tile(
                            [P, D], F32, tag="ysb", name="ysb")

                    f1.add(alloc)
                    for nch in range(2):
                        def mm(gc, cell=cell, sb=sb, nch=nch):
                            if gc == 0:
                                cell["py"] = ps_fill.tile(
                                    [P, CHW], F32, tag="fill", name="fillpy")
                            nc.tensor.matmul(
                                cell["py"], oT[:, gc, sb * P:(sb + 1) * P],
                                wo_sb[:, gc, nch * CHW:(nch + 1) * CHW],
                                start=(gc == 0), stop=(gc == 1))

                        def cp(cell=cell, nch=nch):
                            nc.vector.tensor_copy(
                                out=cell["ysb"][:, nch * CHW:(nch + 1) * CHW],
                                in_=cell["py"])

                        f1.add(lambda mm=mm: mm(0))
                        f1.add(lambda mm=mm: mm(1))
                        f1.add(cp)

                    def out_dma(cell=cell, sb=sb):
                        nc.sync.dma_start(
                            out=y[sb * P:(sb + 1) * P, :], in_=cell["ysb"])

                    f1.add(out_dma)

            _emit_pair_attention(tc, 1, pools, tensors, f1, _outproj_chunk)
            f1.drain()


def _fix_instruction_waits(nc):
    """Some lowered ISA structs (fp32r matmul LDW, DMA pseudo) carry at most
    one sync wait. Normalize: hoist excess waits onto NoOps inserted
    immediately before the instruction in the scheduled stream (same engine,
    so program order preserves the wait semantics)."""
    fixed = 0
    for blk in nc.m.functions[0].blocks:
        insts = blk.instructions
        idx = 0
        while idx < len(insts):
            inst = insts[idx]
            si = getattr(inst, "sync_info", None)
            if si is not None and len(si.on_wait) > 1:
                waits = list(si.on_wait)
                for j, wt in enumerate(waits[:-1]):
                    nop = mybir.InstNoOp(
                        name=f"I-wfix{fixed}-{j}-{inst.name}",
                        engine=inst.engine,
                        sync_info=mybir.SyncInfo(on_wait=[wt], on_update=[]))
                    insts.insert(idx, nop)
                    idx += 1
                inst.sync_info = mybir.SyncInfo(
                    on_wait=[waits[-1]], on_update=list(si.on_update))
                fixed += 1
            idx += 1
    return fixed


def _build():
    global _NC_CACHE
    if _NC_CACHE is None:
        nc = bass.Bass()
        with tile.TileContext(nc) as tc:
            _emit(tc)
        _fix_instruction_waits(nc)
        _NC_CACHE = nc
    return _NC_CACHE


def kernel(x, Wq, Wkv, Wo):
    global LAST_RESULTS
    x = np.asarray(x, dtype=np.float32)
    Wq = np.asarray(Wq, dtype=np.float32)
    Wkv = np.asarray(Wkv, dtype=np.float32)
    Wo = np.asarray(Wo, dtype=np.float32)

    nc = _build()
    bf = ml_dtypes.bfloat16
    in_maps = []
    for c in range(8):
        b, g = divmod(c, 4)
        cs = slice(GC * g, GC * (g + 1))
        in_maps.append({
            "xT": np.ascontiguousarray(x[b].T).astype(bf),
            "wq": np.ascontiguousarray(Wq[:, cs]).astype(bf),
            "wk": np.ascontiguousarray(Wkv[:, 0:D][:, cs]).astype(bf),
            "wv": np.ascontiguousarray(Wkv[:, D:2 * D][:, cs]).astype(bf),
            "wo": np.ascontiguousarray(Wo[cs, :]).astype(bf),
        })

    trace = os.environ.get("ATTN_KERNEL_TRACE", "0") == "1"
    res = run_bass_kernel_spmd(nc, in_maps, list(range(8)), trace=trace)
    LAST_RESULTS = res

    out = np.zeros((B, S, D), dtype=np.float32)
    for c in range(8):
        b = c // 4
        out[b] += res.results[c]["y"]
    return out


if __name__ == "__main__":
    rng = np.random.default_rng(0)
    s = 1.0 / np.sqrt(D)
    inputs = {
        "x": rng.standard_normal((B, S, D), dtype=np.float32),
        "Wq": rng.standard_normal((D, D), dtype=np.float32) * s,
        "Wkv": rng.standard_normal((D, 2 * D), dtype=np.float32) * s,
        "Wo": rng.standard_normal((D, D), dtype=np.float32) * s,
    }
    out = kernel(**inputs)
    print("out", out.shape, out.dtype, float(np.abs(out).mean()))
